# revision 1
# baseline (speedup 1.0000x reference)
"""AttentionBlock (GroupNorm + single-head self-attention + proj + residual)
for Trainium2, 8 NeuronCores, data-parallel over (batch, token-half).

Shapes (hardcoded): x [4, 256, 64, 64] fp32, weights [256, 256] fp32.
Each core handles one (batch b, token-half h): 2048 query tokens against the
full 4096 keys/values of its batch, entirely in SBUF.

Structure per core:
  - The host passes x[b] with the core's query half rotated to columns
    0..2047 (attention is token-order invariant), so the program is SPMD
    with no dynamic offsets.
  - GroupNorm is folded into the QKV weights: of = seff*x + beff, so
    K/Q/V come from raw x with runtime-scaled weights w' = w*diag(seff).
    The beff term maps to: a per-query-constant in the softmax for K
    (drops out exactly, like bk), a per-partition bias on Q, and a constant
    V-channel shift that passes through softmax into the output bias.
  - Scores are computed transposed (S^T = K^T Q, keys on partitions); the
    softmax denominator comes free from a ones-column appended to V^T in
    the PV matmul; exp needs no max-subtraction (scores ~ N(0,1)).
  - Matmuls run in float32r (relaxed fp32, 4x PE rate vs fp32); the
    attention operands (k, q, exp(scores), V) and the QKV production use
    fp16, whose element rounding averages out over the softmax sums and
    whose 2-byte weights get the overlapped LDWEIGHTS path that 4-byte
    operands lack. GroupNorm statistics, softmax normalization, projection
    and residual stay f32r/fp32. Measured end-to-end relative error ~1.2e-4.
    Set f32r=False for exact fp32 (~1e-7, ~3x slower).
"""

import sys

try:
    import concourse.bass as bass  # noqa: F401
except ImportError:
    sys.path.insert(0, "/opt/trn_rl_repo")

import numpy as np

import concourse.bass as bass
import concourse.mybir as mybir
import concourse.tile as tile
from concourse.bass import ts
from concourse.bass_utils import run_bass_kernel_spmd
from concourse.masks import make_identity

FP = mybir.dt.float32
FPR = mybir.dt.float32r
AF = mybir.ActivationFunctionType
ALU = mybir.AluOpType
AX = mybir.AxisListType

P = 128
C = 256
HW = 4096
HALF = 2048
NCH = 2          # channel chunks of 128
NJT = 32         # 128-wide key tiles
NIG = 4          # query i-groups of 512
NCHUNK = 8       # 512-wide token chunks of the full image
GROUPS = 32
GSIZE = C // GROUPS          # 8 channels per group
NELEM = GSIZE * HW           # 32768 elements per group
EPS = 1e-6
SCALE = float(C) ** -0.5     # 0.0625


def _split_waits(nc, max_waits=1):
    """The pinned walrus rejects >1 sync-wait on ctrl instructions; hoist
    excess waits onto preceding NoOps on the same engine (same instruction
    stream, so ordering is preserved)."""
    ctr = 0
    for bb in nc.m.functions[0].blocks:
        out = []
        changed = False
        for inst in bb.instructions:
            si = getattr(inst, "sync_info", None)
            waits = list(si.on_wait) if (si and si.on_wait) else []
            if len(waits) > max_waits:
                changed = True
                head, rest = waits[:-max_waits], waits[-max_waits:]
                for k in range(0, len(head), max_waits):
                    ctr += 1
                    nop = mybir.InstNoOp(name=f"I-wsplit-{ctr}", ins=[], outs=[])
                    nop.engine = inst.engine
                    nop.sync_info = mybir.SyncInfo(
                        on_wait=head[k : k + max_waits], on_update=[]
                    )
                    out.append(nop)
                inst.sync_info = mybir.SyncInfo(
                    on_wait=rest, on_update=list(si.on_update or [])
                )
            out.append(inst)
        if changed:
            bb.instructions = out


def build_nc(split=True, reps=1, f32r=True, pv16=True):
    MD = FPR if f32r else FP          # dtype for matmul-feeding SBUF tiles
    # Attention operands (k, q, u = exp(scores), V) tolerate fp16: 5e-4
    # element rounding perturbs softmax scores by ~5e-4 absolute and averages
    # out over the 4096-term sums (~1e-4 end-to-end), and fp16 stationary
    # operands get the fast (overlapped) LDWEIGHTS path that 4-byte f32r
    # lacks. Production matmuls and the projection stay f32r/fp32.
    MH = mybir.dt.float16 if (f32r and pv16) else (FPR if f32r else FP)

    def mdcast(ap):
        return ap.bitcast(FPR) if f32r else ap

    nc = bass.Bass()
    xf = nc.dram_tensor("xf", [C, HW], FP, kind="ExternalInput")
    wqT = nc.dram_tensor("wqT", [C, C], FP, kind="ExternalInput")
    wkT = nc.dram_tensor("wkT", [C, C], FP, kind="ExternalInput")
    wvT = nc.dram_tensor("wvT", [C, C], FP, kind="ExternalInput")
    wpT = nc.dram_tensor("wpT", [C, C], FP, kind="ExternalInput")
    bq2 = nc.dram_tensor("bq2", [P, NCH], FP, kind="ExternalInput")
    bp2 = nc.dram_tensor("bp2", [P, NCH], FP, kind="ExternalInput")
    gns = nc.dram_tensor("gns", [P, NCH], FP, kind="ExternalInput")
    gnb = nc.dram_tensor("gnb", [P, NCH], FP, kind="ExternalInput")
    gsel = nc.dram_tensor("gsel", [P, P], FP, kind="ExternalInput")
    y = nc.dram_tensor("y", [C, HALF], FP, kind="ExternalOutput")

    with tile.TileContext(nc) as tc:
        with (
            tc.tile_pool(name="wts", bufs=1) as wts,
            tc.tile_pool(name="big", bufs=1) as big,
            tc.tile_pool(name="upool", bufs=6) as upool,
            tc.tile_pool(name="small", bufs=3) as small,
            tc.tile_pool(name="stats", bufs=1) as stats,
            tc.tile_pool(name="outp", bufs=3) as outp,
            tc.tile_pool(name="psA", bufs=4, space="PSUM") as psA,
            tc.tile_pool(name="psAcc", bufs=4, space="PSUM") as psAcc,
        ):
            # ---------------- input image first (critical path), then constants
            xf_sb = big.tile([P, NCH, HW], MD, tag="xf")
            # spread the input-image chunks across engine DMA queues so they
            # land in parallel — the groupnorm stats (and so every matmul)
            # serialize behind the last chunk
            dma_engines = [nc.sync, nc.gpsimd, nc.scalar]
            for o in range(NCH):
                for t8 in range(NCHUNK):
                    eng = dma_engines[(o * NCHUNK + t8) % len(dma_engines)]
                    eng.dma_start(
                        out=xf_sb[:, o, ts(t8, 512)],
                        in_=mdcast(xf[o * P : (o + 1) * P, ts(t8, 512)]),
                    )

            # ---------------- constants + input image ----------------
            w_sb = {}
            for name, dram in (("wqT", wqT), ("wkT", wkT), ("wvT", wvT), ("wpT", wpT)):
                t = wts.tile([P, NCH, C], MD, tag=f"w_{name}")
                nc.sync.dma_start(
                    out=t, in_=mdcast(dram.rearrange("(o p) c -> p o c", p=P))
                )
                w_sb[name] = t
            gsel_sb = wts.tile([P, P], FP, tag="gsel")
            nc.sync.dma_start(out=gsel_sb, in_=gsel[:, :])
            bq_sb = wts.tile([P, NCH], FP, tag="bq")
            nc.sync.dma_start(out=bq_sb, in_=bq2[:, :])
            bp_sb = wts.tile([P, NCH], FP, tag="bp")
            nc.sync.dma_start(out=bp_sb, in_=bp2[:, :])
            gns_sb = wts.tile([P, NCH], FP, tag="gns")
            nc.sync.dma_start(out=gns_sb, in_=gns[:, :])
            gnb_sb = wts.tile([P, NCH], FP, tag="gnb")
            nc.sync.dma_start(out=gnb_sb, in_=gnb[:, :])
            ident_fp = wts.tile([P, P], FP, tag="ident_fp")
            make_identity(nc, ident_fp)
            ident = wts.tile([P, P], MD, tag="ident")
            nc.vector.tensor_copy(ident, ident_fp)

            def xfp(o, sl):  # fp32 view of resident x for exact stats/residual
                return xf_sb[:, o, sl].bitcast(FP) if f32r else xf_sb[:, o, sl]

            # fp16 copy of x for the QKV production matmuls (overlapped LDW)
            if f32r and pv16:
                x16 = big.tile([P, NCH, HW], MH, tag="x16")
                for o in range(NCH):
                    for t8 in range(NCHUNK):
                        nc.vector.tensor_copy(
                            x16[:, o, ts(t8, 512)], xfp(o, ts(t8, 512))
                        )
            else:
                x16 = xf_sb

            for _rep in range(reps):
                # ---------------- phase 1a: GroupNorm statistics ----------------
                sum_cols = stats.tile([P, NCH, NCHUNK], FP, tag="sumc")
                sq_cols = stats.tile([P, NCH, NCHUNK], FP, tag="sqc")
                for t8 in range(NCHUNK):
                    for o in range(NCH):
                        nc.vector.tensor_reduce(
                            out=sum_cols[:, o, t8 : t8 + 1],
                            in_=xfp(o, ts(t8, 512)),
                            axis=AX.X, op=ALU.add,
                        )
                        sq_scr = small.tile([P, 512], FP, tag="sqscr")
                        nc.scalar.activation(
                            out=sq_scr, in_=xfp(o, ts(t8, 512)), func=AF.Square,
                            accum_out=sq_cols[:, o, t8 : t8 + 1],
                        )

                seff = stats.tile([P, NCH], FP, tag="seff")
                beff = stats.tile([P, NCH], FP, tag="beff")
                eps_sb = stats.tile([P, 1], FP, tag="eps")
                nc.vector.memset(eps_sb, EPS)
                for o in range(NCH):
                    part = stats.tile([P, 2], FP, tag=f"part{o}")
                    nc.vector.tensor_reduce(
                        out=part[:, 0:1], in_=sum_cols[:, o], axis=AX.X, op=ALU.add
                    )
                    nc.vector.tensor_reduce(
                        out=part[:, 1:2], in_=sq_cols[:, o], axis=AX.X, op=ALU.add
                    )
                    gps = psA.tile([P, 512], FP, tag="ps512", name="gps")[:, :2]
                    nc.tensor.matmul(gps, lhsT=gsel_sb, rhs=part, start=True, stop=True)
                    mean = stats.tile([P, 1], FP, tag=f"mean{o}")
                    nc.vector.tensor_scalar_mul(mean, gps[:, 0:1], 1.0 / NELEM)
                    ex2 = stats.tile([P, 1], FP, tag=f"ex2{o}")
                    nc.vector.tensor_scalar_mul(ex2, gps[:, 1:2], 1.0 / NELEM)
                    msq = stats.tile([P, 1], FP, tag=f"msq{o}")
                    nc.vector.tensor_mul(msq, mean, mean)
                    var = stats.tile([P, 1], FP, tag=f"var{o}")
                    nc.vector.tensor_tensor(var, ex2, msq, ALU.subtract)
                    # rstd = exp(-0.5 * ln(var + eps)) — stays in the exp table set
                    lnv = stats.tile([P, 1], FP, tag=f"lnv{o}")
                    nc.scalar.activation(out=lnv, in_=var, func=AF.Ln, bias=eps_sb)
                    rstd = stats.tile([P, 1], FP, tag=f"rstd{o}")
                    nc.scalar.activation(out=rstd, in_=lnv, func=AF.Exp, scale=-0.5)
                    nc.vector.tensor_mul(seff[:, o : o + 1], gns_sb[:, o : o + 1], rstd)
                    tmp = stats.tile([P, 1], FP, tag=f"tmp{o}")
                    nc.vector.tensor_mul(tmp, mean, seff[:, o : o + 1])
                    nc.vector.tensor_tensor(
                        beff[:, o : o + 1], gnb_sb[:, o : o + 1], tmp, ALU.subtract
                    )

                # ---------------- phase 1b: fold GN into the weights ----------------
                # bias matvecs on the raw weights first: qb = wq@beff + bq,
                # vb = wv@beff, pvb = wp@vb; then scale wq/wk/wv by seff in place
                def matvec(wname, rhs_sb, out_tile):
                    # plain fp32 matmuls (N=1 is not f32r-legal; cost is trivial)
                    for oo in range(NCH):
                        mv = psA.tile([P, 512], FP, tag="ps512", name="mv")[:, :1]
                        for oi in range(NCH):
                            lhs = w_sb[wname][:, oi, oo * P : (oo + 1) * P]
                            nc.tensor.matmul(
                                mv,
                                lhsT=lhs.bitcast(FP) if f32r else lhs,
                                rhs=rhs_sb[:, oi : oi + 1],
                                start=(oi == 0), stop=(oi == NCH - 1),
                            )
                        nc.vector.tensor_copy(out_tile[:, oo : oo + 1], mv)

                qb = stats.tile([P, NCH], FP, tag="qb")
                vb = stats.tile([P, NCH], FP, tag="vb")
                pvb = stats.tile([P, NCH], FP, tag="pvb")
                ob = stats.tile([P, NCH], FP, tag="ob")
                matvec("wqT", beff, qb)
                nc.vector.tensor_add(qb, qb, bq_sb)
                matvec("wvT", beff, vb)
                matvec("wpT", vb, pvb)
                nc.vector.tensor_add(ob, pvb, bp_sb)

                w2 = {}
                for wname in ("wqT", "wkT", "wvT"):
                    w2[wname] = wts.tile([P, NCH, C], MH, tag=f"w2_{wname}", name=f"w2{wname}")
                    for o in range(NCH):
                        nc.vector.tensor_scalar_mul(
                            w2[wname][:, o], w_sb[wname][:, o], seff[:, o : o + 1]
                        )

                # ---------------- phase 1c: K, V'^T, Q ----------------
                k_sb = big.tile([P, NCH, HW], MH, tag="k")
                v_sb = big.tile([P, NJT, C + 2], MH, tag="v")
                if MH == FPR:
                    nc.vector.memset(v_sb[:, :, C : C + 1].bitcast(FP), 1.0)
                    nc.vector.memset(v_sb[:, :, C + 1 : C + 2].bitcast(FP), 0.0)
                else:
                    nc.vector.memset(v_sb[:, :, C : C + 1], 1.0)
                    nc.vector.memset(v_sb[:, :, C + 1 : C + 2], 0.0)
                for t8 in range(NCHUNK):
                    for oo in range(NCH):
                        ps = psA.tile([P, 512], FP, tag="ps512", name="psk")
                        for oi in range(NCH):
                            nc.tensor.matmul(
                                ps,
                                lhsT=w2["wkT"][:, oi, oo * P : (oo + 1) * P],
                                rhs=x16[:, oi, ts(t8, 512)],
                                start=(oi == 0), stop=(oi == NCH - 1),
                            )
                        if (t8 + oo) % 2 == 0:
                            nc.vector.tensor_copy(
                                out=k_sb[:, oo, ts(t8, 512)], in_=ps
                            )
                        else:
                            nc.scalar.copy(out=k_sb[:, oo, ts(t8, 512)], in_=ps)
                    for jj in range(4):
                        j = t8 * 4 + jj
                        ps = psA.tile([P, 512], FP, tag="ps512", name="psv")[:, :C]
                        for oi in range(NCH):
                            nc.tensor.matmul(
                                ps,
                                lhsT=x16[:, oi, j * P : (j + 1) * P],
                                rhs=w2["wvT"][:, oi],
                                start=(oi == 0), stop=(oi == NCH - 1),
                            )
                        if j % 2 == 0:
                            nc.vector.tensor_copy(out=v_sb[:, j, 0:C], in_=ps)
                        else:
                            nc.scalar.copy(out=v_sb[:, j, 0:C], in_=ps)

                q_sb = big.tile([P, NCH, HALF], MH, tag="q")
                for oo in range(NCH):
                    for i4 in range(4):
                        ps = psA.tile([P, 512], FP, tag="ps512", name="psq")
                        for oi in range(NCH):
                            nc.tensor.matmul(
                                ps,
                                lhsT=w2["wqT"][:, oi, oo * P : (oo + 1) * P],
                                rhs=x16[:, oi, ts(i4, 512)],
                                start=(oi == 0), stop=(oi == NCH - 1),
                            )
                        nc.vector.tensor_scalar_add(
                            q_sb[:, oo, ts(i4, 512)], ps, qb[:, oo : oo + 1]
                        )

                # ---------------- phase 2: attention ----------------
                att_sb = big.tile([P, NCH, HALF], MD, tag="att")
                for g in range(NIG):
                    acc = [
                        psAcc.tile([P, C + 2], FP, tag="acc", name=f"acc{g}_{t}")
                        for t in range(4)
                    ]
                    for j in range(NJT):
                        ps = psA.tile([P, 512], FP, tag="ps512", name="pss")
                        for o in range(NCH):
                            nc.tensor.matmul(
                                ps,
                                lhsT=k_sb[:, o, j * P : (j + 1) * P],
                                rhs=q_sb[:, o, ts(g, 512)],
                                start=(o == 0), stop=(o == NCH - 1),
                            )
                        u = upool.tile([P, 512], MH, tag="u")
                        nc.scalar.activation(out=u, in_=ps, func=AF.Exp, scale=SCALE)
                        for t in range(4):
                            nc.tensor.matmul(
                                acc[t],
                                lhsT=u[:, t * P : (t + 1) * P],
                                rhs=v_sb[:, j],
                                start=(j == 0), stop=(j == NJT - 1),
                            )
                    for t in range(4):
                        rs = small.tile([P, 1], FP, tag="rs")
                        nc.vector.reciprocal(rs, acc[t][:, C : C + 1])
                        asb = small.tile([P, C], MD, tag="asb")
                        nc.vector.tensor_scalar_mul(asb, acc[t][:, 0:C], rs)
                        for o in range(NCH):
                            tps = psAcc.tile([P, P], FP, tag="acc", name="tps")
                            nc.tensor.transpose(
                                tps.bitcast(FPR) if f32r else tps,
                                asb[:, o * P : (o + 1) * P],
                                ident,
                            )
                            col = g * 512 + t * P
                            nc.vector.tensor_copy(
                                out=att_sb[:, o, col : col + P], in_=tps
                            )
                    # projection + residual; the LAST group uses quarter
                    # chunks so its ACT/DVE/DMA drain pipelines instead of
                    # serializing behind PE's final matmul
                    sub = 2 if g == NIG - 1 else 1
                    w_sub = 512 // sub
                    for oo in range(NCH):
                        for s in range(sub):
                            col = g * 512 + s * w_sub
                            ps = psA.tile([P, 512], FP, tag="ps512", name="psp")[
                                :, :w_sub
                            ]
                            for oi in range(NCH):
                                nc.tensor.matmul(
                                    ps,
                                    lhsT=w_sb["wpT"][:, oi, oo * P : (oo + 1) * P],
                                    rhs=att_sb[:, oi, col : col + w_sub],
                                    start=(oi == 0), stop=(oi == NCH - 1),
                                )
                            ot = outp.tile([P, 512], FP, tag="out", name="ot")[
                                :, :w_sub
                            ]
                            nc.scalar.activation(
                                out=ot, in_=ps, func=AF.Identity,
                                bias=ob[:, oo : oo + 1],
                            )
                            nc.vector.tensor_add(
                                ot, ot, xfp(oo, slice(col, col + w_sub))
                            )
                            nc.sync.dma_start(
                                out=y[oo * P : (oo + 1) * P, col : col + w_sub],
                                in_=ot,
                            )

    if split:
        _split_waits(nc)
    return nc


_NC_CACHE = None


def _get_nc():
    global _NC_CACHE
    if _NC_CACHE is None:
        _NC_CACHE = build_nc()
    return _NC_CACHE


def make_in_maps(x, gn_scale, gn_bias, wq, bq, wk, bk, wv, bv, wp, bp):
    B = x.shape[0]
    f32 = np.float32

    def col2(v):  # [256] -> [128, 2], column o = channels o*128..o*128+127
        return np.ascontiguousarray(np.asarray(v, f32).reshape(NCH, P).T)

    wqT = np.ascontiguousarray(np.asarray(wq, f32).T)
    wkT = np.ascontiguousarray(np.asarray(wk, f32).T)
    wvT = np.ascontiguousarray(np.asarray(wv, f32).T)
    wpT = np.ascontiguousarray(np.asarray(wp, f32).T)
    bp2 = col2(np.asarray(wp, f32) @ np.asarray(bv, f32) + np.asarray(bp, f32))
    bq2 = col2(bq)
    gns = col2(gn_scale)
    gnb = col2(gn_bias)
    gsel = np.kron(np.eye(P // GSIZE, dtype=f32), np.ones((GSIZE, GSIZE), f32))

    xr = np.asarray(x, f32).reshape(B, C, HW)
    in_maps = []
    for core in range(8):
        b, h = core // 2, core % 2
        # rotate so this core's query half sits at columns 0..HALF-1
        # (attention/groupnorm are token-order invariant)
        xfb = np.ascontiguousarray(np.roll(xr[b], -h * HALF, axis=1))
        in_maps.append(
            {
                "xf": xfb,
                "wqT": wqT, "wkT": wkT, "wvT": wvT, "wpT": wpT,
                "bq2": bq2, "bp2": bp2, "gns": gns, "gnb": gnb, "gsel": gsel,
            }
        )
    return in_maps


def assemble_out(results, B=4):
    out = np.empty((B, C, HW), np.float32)
    for core in range(8):
        b, h = core // 2, core % 2
        out[b, :, h * HALF : (h + 1) * HALF] = results[core]["y"]
    return out.reshape(B, C, 64, 64)


def kernel(**inputs):
    in_maps = make_in_maps(**inputs)
    try:
        nc = _get_nc()
        res = run_bass_kernel_spmd(nc, in_maps, list(range(8)))
    except Exception:
        # fallback: exact-fp32 variant (4x slower matmuls) after letting a
        # possibly-wedged device recover
        import time as _time

        _time.sleep(90)
        nc = build_nc(f32r=False)
        res = run_bass_kernel_spmd(nc, in_maps, list(range(8)))
    return assemble_out(res.results, B=inputs["x"].shape[0])


if __name__ == "__main__":
    rng = np.random.default_rng(0)
    ins = {
        "x": rng.standard_normal((4, C, 64, 64)).astype(np.float32),
        "gn_scale": np.ones(C, np.float32),
        "gn_bias": np.zeros(C, np.float32),
    }
    for w in ("wq", "wk", "wv", "wp"):
        ins[w] = (rng.standard_normal((C, C)) / 16.0).astype(np.float32)
    for b in ("bq", "bk", "bv", "bp"):
        ins[b] = np.zeros(C, np.float32)
    out = kernel(**ins)
    print(out.shape, out.dtype, float(np.abs(out).mean()))

